# revision 2
# baseline (speedup 1.0000x reference)
"""Trainium2 Bass kernel for nn_AddShift_mp_module (scatter_memory).

Contract: kernel(**inputs) takes the FULL unsharded inputs
(x (32,640,58,58) f32, pad_hv (640,8) i32, idx_identit (128,4) i32,
hout=56, wout=56) and returns the full (out_h, out_v, out_id) tuple,
each (32,128,56,56) f32 — matching reference.reference().

Strategy:
 - Data-parallel over batch: 8 NeuronCores x 4 images each.
 - Reformulate the per-channel shifts as shift-classes: for each distinct
   shift value s, a 0/1 channel-selection matrix (built host-side from the
   runtime pad_hv / idx_identit values, fed as kernel inputs) gathers+sums
   the contributing channels via TensorE matmuls; the spatial shift itself
   is a free-dim offset baked into the rhs access pattern.  All 26 matmuls
   of one output row-chunk accumulate in a single PSUM bank (a zero-weight
   start=True matmul initializes the bank so partial-coverage shifts are
   safe).  PSUM -> SBUF copies and output DMAs overlap PE via Tile.
"""

import os
import numpy as np

# ---- hardcoded problem geometry ----
B, C_IN, HIN, WIN = 32, 640, 58, 58
C_OUT, NK, KC = 128, 5, 5           # KC = contraction chunks of 128 channels
HOUT = WOUT = 56
N_CORES = 8
B_LOC = B // N_CORES                 # 4 images per core
RCH = 8                              # output rows per PSUM chunk
RC = HOUT // RCH                     # 7 row chunks
NPIX = HOUT * WOUT                   # 3136

_PROG_CACHE = {}


def _valid_range(s):
    # output positions where the shifted read index stays inside [0, 58)
    return max(0, -1 - s), min(HOUT, HIN - 1 - s)


def _build_program(shifts_h, shifts_v, mm_dtype_name):
    import concourse.bacc as bacc
    import concourse.mybir as mybir
    import concourse.tile as tile

    f32 = mybir.dt.float32
    mdt = getattr(mybir.dt, mm_dtype_name)
    cast_on_load = mm_dtype_name == "bfloat16"
    x_dt = f32 if cast_on_load else mdt

    NSH, NSV = len(shifts_h), len(shifts_v)

    nc = bacc.Bacc(
        "TRN2", target_bir_lowering=False, debug=False, enable_asserts=False
    )
    x = nc.dram_tensor("x", [B_LOC, KC, 128, HIN, WIN], x_dt, kind="ExternalInput")
    wh = nc.dram_tensor("wh", [NSH * KC, 128, 128], mdt, kind="ExternalInput")
    wv = nc.dram_tensor("wv", [NSV * KC, 128, 128], mdt, kind="ExternalInput")
    wid = nc.dram_tensor("wid", [KC, 128, 128], mdt, kind="ExternalInput")
    oh = nc.dram_tensor("oh", [B_LOC, 128, NPIX], f32, kind="ExternalOutput")
    ov = nc.dram_tensor("ov", [B_LOC, 128, NPIX], f32, kind="ExternalOutput")
    oid = nc.dram_tensor("oid", [B_LOC, 128, NPIX], f32, kind="ExternalOutput")

    with tile.TileContext(nc) as tc:
        with (
            tc.tile_pool(name="wpool", bufs=1) as wpool,
            tc.tile_pool(name="xpool", bufs=2) as xpool,
            tc.tile_pool(name="opool", bufs=2) as opool,
            tc.tile_pool(name="pspool", bufs=8, space="PSUM") as pspool,
        ):
            wht = wpool.tile([128, NSH * KC, 128], mdt, tag="wh")
            wvt = wpool.tile([128, NSV * KC, 128], mdt, tag="wv")
            widt = wpool.tile([128, KC, 128], mdt, tag="wid")
            wzt = wpool.tile([128, 128], mdt, tag="wz")
            nc.sync.dma_start(out=wht[:], in_=wh[:].rearrange("a p c -> p a c"))
            nc.sync.dma_start(out=wvt[:], in_=wv[:].rearrange("a p c -> p a c"))
            nc.sync.dma_start(out=widt[:], in_=wid[:].rearrange("a p c -> p a c"))
            nc.vector.memset(wzt[:], 0.0)

            for b in range(B_LOC):
                xt = xpool.tile([128, KC, HIN, WIN], mdt, tag="x")
                for kc in range(KC):
                    if cast_on_load:
                        nc.gpsimd.dma_start(out=xt[:, kc], in_=x[b, kc])
                    else:
                        nc.sync.dma_start(out=xt[:, kc], in_=x[b, kc])

                # ops[rc] = list of (w_slot, kc, dr0, rcnt, dc0, ccnt, rh0, rw0)
                for out_dram, wt, kind in (
                    (oh, wht, "h"),
                    (ov, wvt, "v"),
                    (oid, widt, "id"),
                ):
                    ops = [[] for _ in range(RC)]
                    if kind == "id":
                        for kc in range(KC):
                            for rc in range(RC):
                                ops[rc].append(
                                    (kc, kc, 0, RCH, 0, WOUT, rc * RCH + 1, 1)
                                )
                    elif kind == "h":
                        for si, s in enumerate(shifts_h):
                            lo, hi = _valid_range(s)
                            if hi <= lo:
                                continue
                            for kc in range(KC):
                                for rc in range(RC):
                                    ops[rc].append(
                                        (si * KC + kc, kc, 0, RCH, lo, hi - lo,
                                         rc * RCH + 1, 1 + s + lo)
                                    )
                    else:
                        for si, s in enumerate(shifts_v):
                            lo, hi = _valid_range(s)
                            for kc in range(KC):
                                for rc in range(RC):
                                    r0 = max(rc * RCH, lo)
                                    r1 = min(rc * RCH + RCH, hi)
                                    if r1 <= r0:
                                        continue
                                    ops[rc].append(
                                        (si * KC + kc, kc, r0 - rc * RCH, r1 - r0,
                                         0, WOUT, r0 + 1 + s, 1)
                                    )

                    pst = [
                        pspool.tile([128, RCH, WOUT], f32, tag="ps", name=f"ps{rc}")
                        for rc in range(RC)
                    ]
                    # zero-init each bank (start=True over the full chunk)
                    for rc in range(RC):
                        nc.tensor.matmul(
                            pst[rc][:, :, :],
                            wzt[:],
                            xt[:, 0, 1:1 + RCH, 1:1 + WOUT],
                            start=True, stop=False, skip_group_check=True,
                        )
                    # emit in (w_slot)-major order so lhsT stays loaded across
                    # the 7 row-chunk matmuls
                    order = sorted(
                        ((rc, i) for rc in range(RC) for i in range(len(ops[rc]))),
                        key=lambda t: (ops[t[0]][t[1]][0], t[0]),
                    )
                    done = [0] * RC
                    for rc, i in order:
                        slot, kc, dr0, rcnt, dc0, ccnt, rh0, rw0 = ops[rc][i]
                        done[rc] += 1
                        nc.tensor.matmul(
                            pst[rc][:, dr0:dr0 + rcnt, dc0:dc0 + ccnt],
                            wt[:, slot, :],
                            xt[:, kc, rh0:rh0 + rcnt, rw0:rw0 + ccnt],
                            start=False, stop=done[rc] == len(ops[rc]),
                            skip_group_check=True,
                        )

                    ot = opool.tile([128, NPIX], f32, tag="o" + kind)
                    for rc in range(RC):
                        nc.any.tensor_copy(
                            ot[:, rc * RCH * WOUT:(rc + 1) * RCH * WOUT],
                            pst[rc][:].rearrange("p a b -> p (a b)"),
                        )
                    nc.sync.dma_start(out=out_dram[b], in_=ot[:])

    nc.compile()
    return nc


def kernel(x, pad_hv, idx_identit, hout, wout):
    x = np.ascontiguousarray(np.asarray(x, dtype=np.float32))
    pad_hv = np.asarray(pad_hv)
    idx_identit = np.asarray(idx_identit)
    assert x.shape == (B, C_IN, HIN, WIN), x.shape
    assert int(hout) == HOUT and int(wout) == WOUT

    mm_dtype = os.environ.get("KERNEL_MM_DTYPE", "bfloat16")

    shifts_h = sorted({int(v) for v in pad_hv[:, 0:4].ravel()})
    shifts_v = sorted({int(v) for v in pad_hv[:, 4:8].ravel()})

    key = (tuple(shifts_h), tuple(shifts_v), mm_dtype)
    if key not in _PROG_CACHE:
        _PROG_CACHE[key] = _build_program(shifts_h, shifts_v, mm_dtype)
    nc = _PROG_CACHE[key]

    # ---- host-side weight build (counts multiplicity over groups) ----
    NSH, NSV = len(shifts_h), len(shifts_v)
    WH = np.zeros((NSH * KC, 128, 128), np.float32)
    WV = np.zeros((NSV * KC, 128, 128), np.float32)
    WID = np.zeros((KC, 128, 128), np.float32)
    sh_idx = {s: i for i, s in enumerate(shifts_h)}
    sv_idx = {s: i for i, s in enumerate(shifts_v)}
    for c in range(C_IN):
        kc, p, co = c // 128, c % 128, c // NK
        for g in range(4):
            WH[sh_idx[int(pad_hv[c, g])] * KC + kc, p, co] += 1.0
            WV[sv_idx[int(pad_hv[c, 4 + g])] * KC + kc, p, co] += 1.0
    for co in range(C_OUT):
        for g in range(4):
            c = int(idx_identit[co, g])
            WID[c // 128, c % 128, co] += 1.0

    if mm_dtype == "bfloat16":
        import ml_dtypes
        WH = WH.astype(ml_dtypes.bfloat16)
        WV = WV.astype(ml_dtypes.bfloat16)
        WID = WID.astype(ml_dtypes.bfloat16)

    xr = x.reshape(B, KC, 128, HIN, WIN)
    in_maps = [
        {
            "x": xr[i * B_LOC:(i + 1) * B_LOC],
            "wh": WH,
            "wv": WV,
            "wid": WID,
        }
        for i in range(N_CORES)
    ]

    from concourse.bass_utils import run_bass_kernel_spmd

    res = run_bass_kernel_spmd(nc, in_maps, core_ids=list(range(N_CORES)))

    out_h = np.concatenate([r["oh"] for r in res.results]).reshape(
        B, C_OUT, HOUT, WOUT
    )
    out_v = np.concatenate([r["ov"] for r in res.results]).reshape(
        B, C_OUT, HOUT, WOUT
    )
    out_id = np.concatenate([r["oid"] for r in res.results]).reshape(
        B, C_OUT, HOUT, WOUT
    )
    return out_h, out_v, out_id


# revision 6
# speedup vs baseline: 27568.4741x; 27568.4741x over previous
"""Trainium2 Bass kernel for nn_AddShift_mp_module (scatter_memory).

Contract: kernel(**inputs) takes the FULL unsharded inputs
(x (32,640,58,58) f32, pad_hv (640,8) i32, idx_identit (128,4) i32,
hout=56, wout=56) and returns the full (out_h, out_v, out_id) tuple,
each (32,128,56,56) f32 — matching reference.reference().

Strategy:
 - Data-parallel over batch: 8 NeuronCores x 4 images each.
 - Reformulate the per-channel shifts as shift-classes: for each distinct
   shift value s, a 0/1 channel-selection matrix (built host-side from the
   runtime pad_hv / idx_identit values, fed as kernel inputs) gathers+sums
   the contributing channels via TensorE matmuls; the spatial shift itself
   is a free-dim offset baked into the rhs access pattern.  All 26 matmuls
   of one output row-chunk accumulate in a single PSUM bank (a zero-weight
   start=True matmul initializes the bank so partial-coverage shifts are
   safe).  PSUM -> SBUF copies and output DMAs overlap PE via Tile.
"""

import os
import numpy as np

# ---- hardcoded problem geometry ----
B, C_IN, HIN, WIN = 32, 640, 58, 58
C_OUT, NK, KC = 128, 5, 5           # KC = contraction chunks of 128 channels
HOUT = WOUT = 56
N_CORES = 8
B_LOC = B // N_CORES                 # 4 images per core
RCH = 8                              # output rows per PSUM chunk
RC = HOUT // RCH                     # 7 row chunks
NPIX = HOUT * WOUT                   # 3136

_PROG_CACHE = {}


def _valid_range(s):
    # output positions where the shifted read index stays inside [0, 58)
    return max(0, -1 - s), min(HOUT, HIN - 1 - s)


def _build_program(shifts_h, shifts_v, mm_dtype_name):
    import concourse.bacc as bacc
    import concourse.mybir as mybir
    import concourse.tile as tile

    f32 = mybir.dt.float32
    mdt = getattr(mybir.dt, mm_dtype_name)
    cast_on_load = mm_dtype_name in ("bfloat16", "float16")
    x_dt = f32 if cast_on_load else mdt

    NSH, NSV = len(shifts_h), len(shifts_v)

    nc = bacc.Bacc(
        "TRN2", target_bir_lowering=False, debug=False, enable_asserts=False
    )
    x = nc.dram_tensor("x", [B_LOC, KC, 128, HIN, WIN], x_dt, kind="ExternalInput")
    wh = nc.dram_tensor("wh", [NSH * KC, 128, 128], mdt, kind="ExternalInput")
    wv = nc.dram_tensor("wv", [NSV * KC, 128, 128], mdt, kind="ExternalInput")
    wid = nc.dram_tensor("wid", [KC, 128, 128], mdt, kind="ExternalInput")
    oh = nc.dram_tensor("oh", [B_LOC, 128, NPIX], f32, kind="ExternalOutput")
    ov = nc.dram_tensor("ov", [B_LOC, 128, NPIX], f32, kind="ExternalOutput")
    oid = nc.dram_tensor("oid", [B_LOC, 128, NPIX], f32, kind="ExternalOutput")

    # f32r tiles are 2x bf16 size; drop buffering to fit the SBUF budget
    xbufs = 2 if cast_on_load else 1
    obufs = 2 if cast_on_load else 1

    with tile.TileContext(nc) as tc:
        with (
            tc.tile_pool(name="wpool", bufs=1) as wpool,
            tc.tile_pool(name="xpool", bufs=xbufs) as xpool,
            tc.tile_pool(name="opool", bufs=obufs) as opool,
            tc.tile_pool(name="pspool", bufs=8, space="PSUM") as pspool,
        ):
            wht = wpool.tile([128, NSH * KC, 128], mdt, tag="wh")
            wvt = wpool.tile([128, NSV * KC, 128], mdt, tag="wv")
            widt = wpool.tile([128, KC, 128], mdt, tag="wid")
            wzt = wpool.tile([128, 128], mdt, tag="wz")
            nc.sync.dma_start(out=wht[:], in_=wh[:].rearrange("a p c -> p a c"))
            nc.sync.dma_start(out=wvt[:], in_=wv[:].rearrange("a p c -> p a c"))
            nc.sync.dma_start(out=widt[:], in_=wid[:].rearrange("a p c -> p a c"))
            nc.vector.memset(wzt[:], 0.0)

            for b in range(B_LOC):
                xt = xpool.tile([128, KC, HIN, WIN], mdt, tag="x")
                for kc in range(KC):
                    if cast_on_load:
                        nc.gpsimd.dma_start(out=xt[:, kc], in_=x[b, kc])
                    else:
                        nc.sync.dma_start(out=xt[:, kc], in_=x[b, kc])

                # ops[rc] = list of (w_slot, kc, dr0, rcnt, dc0, ccnt, rh0, rw0)
                for out_dram, wt, kind in (
                    (oh, wht, "h"),
                    (ov, wvt, "v"),
                    (oid, widt, "id"),
                ):
                    ops = [[] for _ in range(RC)]
                    if kind == "id":
                        for kc in range(KC):
                            for rc in range(RC):
                                ops[rc].append(
                                    (kc, kc, 0, RCH, 0, WOUT, rc * RCH + 1, 1)
                                )
                    elif kind == "h":
                        for si, s in enumerate(shifts_h):
                            lo, hi = _valid_range(s)
                            if hi <= lo:
                                continue
                            for kc in range(KC):
                                for rc in range(RC):
                                    ops[rc].append(
                                        (si * KC + kc, kc, 0, RCH, lo, hi - lo,
                                         rc * RCH + 1, 1 + s + lo)
                                    )
                    else:
                        for si, s in enumerate(shifts_v):
                            lo, hi = _valid_range(s)
                            for kc in range(KC):
                                for rc in range(RC):
                                    r0 = max(rc * RCH, lo)
                                    r1 = min(rc * RCH + RCH, hi)
                                    if r1 <= r0:
                                        continue
                                    ops[rc].append(
                                        (si * KC + kc, kc, r0 - rc * RCH, r1 - r0,
                                         0, WOUT, r0 + 1 + s, 1)
                                    )

                    pst = [
                        pspool.tile([128, RCH, WOUT], f32, tag="ps", name=f"ps{rc}")
                        for rc in range(RC)
                    ]
                    # zero-init each bank (start=True over the full chunk)
                    for rc in range(RC):
                        nc.tensor.matmul(
                            pst[rc][:, :, :],
                            wzt[:],
                            xt[:, 0, 1:1 + RCH, 1:1 + WOUT],
                            start=True, stop=False, skip_group_check=True,
                        )
                    # emit in (w_slot)-major order so lhsT stays loaded across
                    # the 7 row-chunk matmuls
                    order = sorted(
                        ((rc, i) for rc in range(RC) for i in range(len(ops[rc]))),
                        key=lambda t: (ops[t[0]][t[1]][0], t[0]),
                    )
                    done = [0] * RC
                    for rc, i in order:
                        slot, kc, dr0, rcnt, dc0, ccnt, rh0, rw0 = ops[rc][i]
                        done[rc] += 1
                        nc.tensor.matmul(
                            pst[rc][:, dr0:dr0 + rcnt, dc0:dc0 + ccnt],
                            wt[:, slot, :],
                            xt[:, kc, rh0:rh0 + rcnt, rw0:rw0 + ccnt],
                            start=False, stop=done[rc] == len(ops[rc]),
                            skip_group_check=True,
                        )

                    ot = opool.tile([128, NPIX], f32, tag="o" + kind)
                    for rc in range(RC):
                        nc.any.tensor_copy(
                            ot[:, rc * RCH * WOUT:(rc + 1) * RCH * WOUT],
                            pst[rc][:].rearrange("p a b -> p (a b)"),
                        )
                    nc.sync.dma_start(out=out_dram[b], in_=ot[:])

    nc.compile()
    return nc


def _build_weights(pad_hv, idx_identit, shifts_h, shifts_v, mm_dtype):
    NSH, NSV = len(shifts_h), len(shifts_v)
    WH = np.zeros((NSH * KC, 128, 128), np.float32)
    WV = np.zeros((NSV * KC, 128, 128), np.float32)
    WID = np.zeros((KC, 128, 128), np.float32)
    sh_idx = {s: i for i, s in enumerate(shifts_h)}
    sv_idx = {s: i for i, s in enumerate(shifts_v)}
    for c in range(C_IN):
        kc, p, co = c // 128, c % 128, c // NK
        for g in range(4):
            WH[sh_idx[int(pad_hv[c, g])] * KC + kc, p, co] += 1.0
            WV[sv_idx[int(pad_hv[c, 4 + g])] * KC + kc, p, co] += 1.0
    for co in range(C_OUT):
        for g in range(4):
            c = int(idx_identit[co, g])
            WID[c // 128, c % 128, co] += 1.0
    if mm_dtype == "bfloat16":
        import ml_dtypes
        wnp = ml_dtypes.bfloat16
    elif mm_dtype == "float16":
        wnp = np.float16
    else:
        wnp = np.float32
    return WH.astype(wnp), WV.astype(wnp), WID.astype(wnp)


def kernel(x, pad_hv, idx_identit, hout, wout):
    x = np.ascontiguousarray(np.asarray(x, dtype=np.float32))
    pad_hv = np.asarray(pad_hv)
    idx_identit = np.asarray(idx_identit)
    assert x.shape == (B, C_IN, HIN, WIN), x.shape
    assert int(hout) == HOUT and int(wout) == WOUT

    mm_dtype = os.environ.get("KERNEL_MM_DTYPE", "float16")

    shifts_h = sorted({int(v) for v in pad_hv[:, 0:4].ravel()})
    shifts_v = sorted({int(v) for v in pad_hv[:, 4:8].ravel()})

    key = (tuple(shifts_h), tuple(shifts_v), mm_dtype)
    if key not in _PROG_CACHE:
        _PROG_CACHE[key] = _build_program(shifts_h, shifts_v, mm_dtype)
    nc = _PROG_CACHE[key]

    WH, WV, WID = _build_weights(pad_hv, idx_identit, shifts_h, shifts_v, mm_dtype)

    xr = x.reshape(B, KC, 128, HIN, WIN)
    in_maps = [
        {
            "x": xr[i * B_LOC:(i + 1) * B_LOC],
            "wh": WH,
            "wv": WV,
            "wid": WID,
        }
        for i in range(N_CORES)
    ]

    from concourse.bass_utils import run_bass_kernel_spmd

    res = run_bass_kernel_spmd(nc, in_maps, core_ids=list(range(N_CORES)))

    out_h = np.concatenate([r["oh"] for r in res.results]).reshape(
        B, C_OUT, HOUT, WOUT
    )
    out_v = np.concatenate([r["ov"] for r in res.results]).reshape(
        B, C_OUT, HOUT, WOUT
    )
    out_id = np.concatenate([r["oid"] for r in res.results]).reshape(
        B, C_OUT, HOUT, WOUT
    )
    return out_h, out_v, out_id


# revision 9
# speedup vs baseline: 34882.4449x; 1.2653x over previous
"""Trainium2 Bass kernel for nn_AddShift_mp_module (scatter_memory).

Contract: kernel(**inputs) takes the FULL unsharded inputs
(x (32,640,58,58) f32, pad_hv (640,8) i32, idx_identit (128,4) i32,
hout=56, wout=56) and returns the full (out_h, out_v, out_id) tuple,
each (32,128,56,56) f32 — matching reference.reference().

Strategy:
 - Data-parallel over batch: 8 NeuronCores x 4 images each.
 - Reformulate the per-channel shifts as shift-classes: for each distinct
   shift value s, a 0/1 channel-selection matrix (built host-side from the
   runtime pad_hv / idx_identit values, fed as kernel inputs) gathers+sums
   the contributing channels via TensorE matmuls; the spatial shift itself
   is a free-dim offset baked into the rhs access pattern.  All 26 matmuls
   of one output row-chunk accumulate in a single PSUM bank (a zero-weight
   start=True matmul initializes the bank so partial-coverage shifts are
   safe).  PSUM -> SBUF copies and output DMAs overlap PE via Tile.
"""

import os
import numpy as np

# ---- hardcoded problem geometry ----
B, C_IN, HIN, WIN = 32, 640, 58, 58
C_OUT, NK, KC = 128, 5, 5           # KC = contraction chunks of 128 channels
HOUT = WOUT = 56
N_CORES = 8
B_LOC = B // N_CORES                 # 4 images per core
RCH = 8                              # output rows per PSUM chunk
RC = HOUT // RCH                     # 7 row chunks
NPIX = HOUT * WOUT                   # 3136

_PROG_CACHE = {}


def _valid_range(s):
    # output positions where the shifted read index stays inside [0, 58)
    return max(0, -1 - s), min(HOUT, HIN - 1 - s)


def _build_program(shifts_h, shifts_v, mm_dtype_name, dve_h=(), dve_v=()):
    import concourse.bacc as bacc
    import concourse.mybir as mybir
    import concourse.tile as tile

    f32 = mybir.dt.float32
    mdt = getattr(mybir.dt, mm_dtype_name)
    cast_on_load = mm_dtype_name in ("bfloat16", "float16")
    x_dt = f32 if cast_on_load else mdt

    NSH, NSV = len(shifts_h), len(shifts_v)

    nc = bacc.Bacc(
        "TRN2", target_bir_lowering=False, debug=False, enable_asserts=False
    )
    x = nc.dram_tensor("x", [B_LOC, 128, KC, HIN, WIN], x_dt, kind="ExternalInput")
    mh = nc.dram_tensor("mh", [128, NSH * KC], f32, kind="ExternalInput")
    mv = nc.dram_tensor("mv", [128, NSV * KC], f32, kind="ExternalInput")
    wh = nc.dram_tensor("wh", [NSH * KC, 128, 128], mdt, kind="ExternalInput")
    wv = nc.dram_tensor("wv", [NSV * KC, 128, 128], mdt, kind="ExternalInput")
    wid = nc.dram_tensor("wid", [KC, 128, 128], mdt, kind="ExternalInput")
    oh = nc.dram_tensor("oh", [B_LOC, 128, NPIX], f32, kind="ExternalOutput")
    ov = nc.dram_tensor("ov", [B_LOC, 128, NPIX], f32, kind="ExternalOutput")
    oid = nc.dram_tensor("oid", [B_LOC, 128, NPIX], f32, kind="ExternalOutput")

    # f32r tiles are 2x bf16 size; drop buffering to fit the SBUF budget
    xbufs = 2 if cast_on_load else 1
    obufs = 2 if cast_on_load else 1

    with tile.TileContext(nc) as tc:
        with (
            tc.tile_pool(name="wpool", bufs=1) as wpool,
            tc.tile_pool(name="xpool", bufs=xbufs) as xpool,
            tc.tile_pool(name="opool", bufs=obufs) as opool,
            tc.tile_pool(name="pspool", bufs=8, space="PSUM") as pspool,
        ):
            wht = wpool.tile([128, NSH * KC, 128], mdt, tag="wh")
            wvt = wpool.tile([128, NSV * KC, 128], mdt, tag="wv")
            widt = wpool.tile([128, KC, 128], mdt, tag="wid")
            wzt = wpool.tile([128, 128], mdt, tag="wz")
            mht = wpool.tile([128, NSH * KC], f32, tag="mh")
            mvt = wpool.tile([128, NSV * KC], f32, tag="mv")
            nc.sync.dma_start(out=wht[:], in_=wh[:].rearrange("a p c -> p a c"))
            nc.sync.dma_start(out=wvt[:], in_=wv[:].rearrange("a p c -> p a c"))
            nc.sync.dma_start(out=widt[:], in_=wid[:].rearrange("a p c -> p a c"))
            nc.sync.dma_start(out=mht[:], in_=mh[:])
            nc.sync.dma_start(out=mvt[:], in_=mv[:])
            nc.vector.memset(wzt[:], 0.0)

            for b in range(B_LOC):
                xt = xpool.tile([128, KC, HIN, WIN], mdt, tag="x")
                if cast_on_load:
                    nc.gpsimd.dma_start(out=xt[:], in_=x[b])
                else:
                    nc.sync.dma_start(out=xt[:], in_=x[b])

                # ops[rc] = list of (w_slot, kc, dr0, rcnt, dc0, ccnt, rh0, rw0)
                for out_dram, wt, kind in (
                    (oh, wht, "h"),
                    (ov, wvt, "v"),
                    (oid, widt, "id"),
                ):
                    if (kind == "h" and b in dve_h) or (kind == "v" and b in dve_v):
                        _emit_dve_branch(
                            nc, opool, out_dram, xt,
                            mht if kind == "h" else mvt,
                            shifts_h if kind == "h" else shifts_v,
                            kind, b, f32,
                        )
                        continue
                    ops = [[] for _ in range(RC)]
                    if kind == "id":
                        for kc in range(KC):
                            for rc in range(RC):
                                ops[rc].append(
                                    (kc, kc, 0, RCH, 0, WOUT, rc * RCH + 1, 1)
                                )
                    elif kind == "h":
                        for si, s in enumerate(shifts_h):
                            lo, hi = _valid_range(s)
                            if hi <= lo:
                                continue
                            for kc in range(KC):
                                for rc in range(RC):
                                    ops[rc].append(
                                        (si * KC + kc, kc, 0, RCH, lo, hi - lo,
                                         rc * RCH + 1, 1 + s + lo)
                                    )
                    else:
                        for si, s in enumerate(shifts_v):
                            lo, hi = _valid_range(s)
                            for kc in range(KC):
                                for rc in range(RC):
                                    r0 = max(rc * RCH, lo)
                                    r1 = min(rc * RCH + RCH, hi)
                                    if r1 <= r0:
                                        continue
                                    ops[rc].append(
                                        (si * KC + kc, kc, r0 - rc * RCH, r1 - r0,
                                         0, WOUT, r0 + 1 + s, 1)
                                    )

                    pst = [
                        pspool.tile([128, RCH, WOUT], f32, tag="ps", name=f"ps{rc}")
                        for rc in range(RC)
                    ]
                    # emit in (w_slot)-major order so lhsT stays loaded across
                    # the 7 row-chunk matmuls
                    order = sorted(
                        ((rc, i) for rc in range(RC) for i in range(len(ops[rc]))),
                        key=lambda t: (ops[t[0]][t[1]][0], t[0]),
                    )
                    # zero-init only banks whose first emitted op is partial
                    first = {}
                    for rc, i in order:
                        first.setdefault(rc, ops[rc][i])
                    for rc in range(RC):
                        _, _, dr0, rcnt, dc0, ccnt, _, _ = first[rc]
                        if not (dr0 == 0 and rcnt == RCH and dc0 == 0 and ccnt == WOUT):
                            nc.tensor.matmul(
                                pst[rc][:, :, :],
                                wzt[:],
                                xt[:, 0, 1:1 + RCH, 1:1 + WOUT],
                                start=True, stop=False, skip_group_check=True,
                            )
                            first[rc] = None
                    done = [0] * RC
                    for rc, i in order:
                        slot, kc, dr0, rcnt, dc0, ccnt, rh0, rw0 = ops[rc][i]
                        done[rc] += 1
                        nc.tensor.matmul(
                            pst[rc][:, dr0:dr0 + rcnt, dc0:dc0 + ccnt],
                            wt[:, slot, :],
                            xt[:, kc, rh0:rh0 + rcnt, rw0:rw0 + ccnt],
                            start=first[rc] is not None and done[rc] == 1,
                            stop=done[rc] == len(ops[rc]),
                            skip_group_check=True,
                        )

                    ot = opool.tile([128, NPIX], f32, tag="o" + kind)
                    for rc in range(RC):
                        nc.scalar.copy(
                            ot[:, rc * RCH * WOUT:(rc + 1) * RCH * WOUT],
                            pst[rc][:].rearrange("p a b -> p (a b)"),
                        )
                    nc.sync.dma_start(out=out_dram[b], in_=ot[:])

    nc.compile()
    return nc


def _emit_dve_branch(nc, opool, out_dram, xt, mt, shifts, kind, b, f32):
    import concourse.mybir as mybir

    mult, add = mybir.AluOpType.mult, mybir.AluOpType.add
    ot = opool.tile([128, HOUT, WOUT], f32, tag="dve" + kind, name=f"dve{kind}{b}")
    first = True
    for si, s in enumerate(shifts):
        lo, hi = _valid_range(s)
        if hi <= lo:
            continue
        for kc in range(KC):
            sc = mt[:, si * KC + kc:si * KC + kc + 1]
            if kind == "h":
                src = xt[:, kc, 1:1 + HOUT, 1 + s + lo:1 + s + hi]
                dst = ot[:, :, lo:hi]
            else:
                src = xt[:, kc, 1 + s + lo:1 + s + hi, 1:1 + WOUT]
                dst = ot[:, lo:hi, :]
            if first:
                assert lo == 0 and hi == HOUT, "first shift must be full-coverage"
                nc.vector.tensor_scalar(dst, src, sc, None, op0=mult)
                first = False
            else:
                nc.vector.scalar_tensor_tensor(dst, src, sc, dst, op0=mult, op1=add)
    nc.sync.dma_start(out=out_dram[b], in_=ot[:].rearrange("p a b -> p (a b)"))


def _build_weights(pad_hv, idx_identit, shifts_h, shifts_v, mm_dtype):
    # k-lattice layout: xt partition p, lane k holds channel 5*p + k
    NSH, NSV = len(shifts_h), len(shifts_v)
    WH = np.zeros((NSH * KC, 128, 128), np.float32)
    WV = np.zeros((NSV * KC, 128, 128), np.float32)
    WID = np.zeros((KC, 128, 128), np.float32)
    MH = np.zeros((128, NSH * KC), np.float32)
    MV = np.zeros((128, NSV * KC), np.float32)
    sh_idx = {s: i for i, s in enumerate(shifts_h)}
    sv_idx = {s: i for i, s in enumerate(shifts_v)}
    for c in range(C_IN):
        co, kc = divmod(c, NK)
        for g in range(4):
            WH[sh_idx[int(pad_hv[c, g])] * KC + kc, co, co] += 1.0
            WV[sv_idx[int(pad_hv[c, 4 + g])] * KC + kc, co, co] += 1.0
            MH[co, sh_idx[int(pad_hv[c, g])] * KC + kc] += 1.0
            MV[co, sv_idx[int(pad_hv[c, 4 + g])] * KC + kc] += 1.0
    for co in range(C_OUT):
        for g in range(4):
            c = int(idx_identit[co, g])
            WID[c % NK, c // NK, co] += 1.0
    if mm_dtype == "bfloat16":
        import ml_dtypes
        wnp = ml_dtypes.bfloat16
    elif mm_dtype == "float16":
        wnp = np.float16
    else:
        wnp = np.float32
    return WH.astype(wnp), WV.astype(wnp), WID.astype(wnp), MH, MV


def kernel(x, pad_hv, idx_identit, hout, wout):
    x = np.ascontiguousarray(np.asarray(x, dtype=np.float32))
    pad_hv = np.asarray(pad_hv)
    idx_identit = np.asarray(idx_identit)
    assert x.shape == (B, C_IN, HIN, WIN), x.shape
    assert int(hout) == HOUT and int(wout) == WOUT

    mm_dtype = os.environ.get("KERNEL_MM_DTYPE", "float16")

    # widest-coverage shift first: the first emitted matmul per PSUM bank
    # then covers the full chunk and can carry start=True (no zero-init)
    cov = lambda s: _valid_range(s)[0] - _valid_range(s)[1]
    shifts_h = sorted({int(v) for v in pad_hv[:, 0:4].ravel()}, key=cov)
    shifts_v = sorted({int(v) for v in pad_hv[:, 4:8].ravel()}, key=cov)

    dve_h = tuple(
        int(v) for v in os.environ.get("KERNEL_DVE_H", "1,3").split(",") if v != ""
    )
    dve_v = tuple(
        int(v) for v in os.environ.get("KERNEL_DVE_V", "").split(",") if v != ""
    )
    key = (tuple(shifts_h), tuple(shifts_v), mm_dtype, dve_h, dve_v)
    if key not in _PROG_CACHE:
        _PROG_CACHE[key] = _build_program(
            shifts_h, shifts_v, mm_dtype, dve_h=dve_h, dve_v=dve_v
        )
    nc = _PROG_CACHE[key]

    WH, WV, WID, MH, MV = _build_weights(
        pad_hv, idx_identit, shifts_h, shifts_v, mm_dtype
    )

    xr = x.reshape(B, 128, KC, HIN, WIN)
    in_maps = [
        {
            "x": xr[i * B_LOC:(i + 1) * B_LOC],
            "wh": WH,
            "wv": WV,
            "wid": WID,
            "mh": MH,
            "mv": MV,
        }
        for i in range(N_CORES)
    ]

    from concourse.bass_utils import run_bass_kernel_spmd

    res = run_bass_kernel_spmd(nc, in_maps, core_ids=list(range(N_CORES)))

    out_h = np.concatenate([r["oh"] for r in res.results]).reshape(
        B, C_OUT, HOUT, WOUT
    )
    out_v = np.concatenate([r["ov"] for r in res.results]).reshape(
        B, C_OUT, HOUT, WOUT
    )
    out_id = np.concatenate([r["oid"] for r in res.results]).reshape(
        B, C_OUT, HOUT, WOUT
    )
    return out_h, out_v, out_id
